# revision 2
# baseline (speedup 1.0000x reference)
"""Trainium2 Bass kernel for BasicMGU (nn_BasicMGU_53386443489965).

Math (per reference):
    xz = x @ W_k ; xh = x @ W_u
    f_t = sigmoid(xz_t + h @ W_r + b_r)
    c_t = tanh(xh_t + (h*f_t) @ W_ur + b_ur)
    h   = (1-f_t)*h + f_t*c_t        -> return final h  [B, U]

Sharding: data-parallel over batch across 8 cores (B=64 -> 8 per core),
weights replicated.

Per-core design (v2):
  Phase 1 (projections): two bf16 GEMMs writing bf16 xz/xh slabs DIRECTLY
  into SBUF (no DRAM roundtrip, no per-chunk slab DMA).  PSUM drains split
  between ACT and DVE so neither serializes the GEMM stream.
  Phase 2 (recurrence): state kept transposed hT [U(part), B(free)].
  Both per-step matmuls run weight-stationary (lhsT = 128x128 bf16 weight
  tile, rhs = state in bf16, N=BL=8).  PSUM tiles are initialized with the
  step's xz/xh slice via a tiny identity-weight matmul on the PE (keeps the
  near-saturated DVE out of it).  Per-chunk slab slices are addressed with
  For_i register offsets straight into the SBUF slabs.
  Accumulation groups are kept consecutive per PSUM slice - interleaving
  groups gives wrong results on HW.
"""

import os
import sys
import types

sys.path.insert(0, "/opt/trn_rl_repo")

import numpy as np
import ml_dtypes

import concourse.bass as bass
import concourse.mybir as mybir
import concourse.tile as tile
from concourse import bacc
from concourse.bass_utils import run_bass_kernel_spmd

B, T, D, U = 64, 1024, 512, 512
NCORES = 8
BL = B // NCORES          # batch per core
S = int(os.environ.get("MGU_S", 64))  # recurrence steps per hw-loop iteration
KC = D // 128             # contraction chunks
MC = U // 128             # output-unit chunks
PCOLS = S * BL            # projection (t,b) columns per block == chunk width
NBLK = T * BL // PCOLS

F32 = mybir.dt.float32
BF16 = mybir.dt.bfloat16

LAST_EXEC_NS = None


def _install_trace_shim():
    """Make `antenv.axon_hooks` importable so trace=True degrades gracefully
    (and, where the axon .so is present, actually captures NTFF profiles)."""
    if "antenv.axon_hooks" in sys.modules:
        return
    mod = types.ModuleType("antenv.axon_hooks")
    holder = [None]
    mod.set_axon_ntff_profile_hook = lambda h: holder.__setitem__(0, h)
    mod.get_axon_ntff_profile_hook = lambda: holder[0]
    sys.modules["antenv.axon_hooks"] = mod
    try:
        if "/root/.axon_site" not in sys.path:
            sys.path.append("/root/.axon_site")
        from trn_agent_boot.trn_boot import _ntff_profile_via_ctypes

        hook = _ntff_profile_via_ctypes("/opt/axon/libaxon_pjrt.so")
        if hook is not None:
            mod.set_axon_ntff_profile_hook(hook)
    except Exception:
        pass


if os.environ.get("MGU_LDWOPT"):
    import concourse.bass_utils as _bu

    _orig_run_command = _bu.run_command

    def _run_command_ldwopt(argv, **kw):
        argv = [
            a.replace("--enable-ldw-opt=false", "--enable-ldw-opt=true")
            for a in argv
        ]
        return _orig_run_command(argv, **kw)

    _bu.run_command = _run_command_ldwopt


def _build():
    nc = bacc.Bacc("TRN2")

    nch = T // S

    xT = nc.dram_tensor("xT", [D, T * BL], BF16, kind="ExternalInput")
    Wk = nc.dram_tensor("Wk", [D, U], BF16, kind="ExternalInput")
    Wu = nc.dram_tensor("Wu", [D, U], BF16, kind="ExternalInput")
    Wr = nc.dram_tensor("Wr", [U, U], BF16, kind="ExternalInput")
    Wur = nc.dram_tensor("Wur", [U, U], BF16, kind="ExternalInput")
    br = nc.dram_tensor("br", [U], F32, kind="ExternalInput")
    bur = nc.dram_tensor("bur", [U], F32, kind="ExternalInput")
    ident = nc.dram_tensor("ident", [128, 128], BF16, kind="ExternalInput")
    hT_out = nc.dram_tensor("hT_out", [128, MC, BL], F32, kind="ExternalOutput")

    ID = mybir.ActivationFunctionType.Identity
    SIG = mybir.ActivationFunctionType.Sigmoid
    TANH = mybir.ActivationFunctionType.Tanh

    with tile.TileContext(nc) as tc:
        with tc.tile_pool(name="consts", bufs=1) as consts:
            Wk_sb = consts.tile([128, KC, U], BF16)
            nc.sync.dma_start(Wk_sb, Wk[:, :].rearrange("(c p) u -> p c u", p=128))
            Wu_sb = consts.tile([128, KC, U], BF16)
            nc.sync.dma_start(Wu_sb, Wu[:, :].rearrange("(c p) u -> p c u", p=128))
            Wr_sb = consts.tile([128, MC, U], BF16)
            nc.sync.dma_start(Wr_sb, Wr[:, :].rearrange("(c p) u -> p c u", p=128))
            Wur_sb = consts.tile([128, MC, U], BF16)
            nc.sync.dma_start(Wur_sb, Wur[:, :].rearrange("(c p) u -> p c u", p=128))
            br_sb = consts.tile([128, MC], F32)
            nc.sync.dma_start(br_sb, br[:].rearrange("(c p) -> p c", p=128))
            bur_sb = consts.tile([128, MC], F32)
            nc.sync.dma_start(bur_sb, bur[:].rearrange("(c p) -> p c", p=128))
            id_sb = consts.tile([128, 128], BF16)
            nc.sync.dma_start(id_sb, ident[:, :])

            # bf16 step-input slabs resident in SBUF: [p, m, (t b)]
            xz_slab = consts.tile([128, MC, T * BL], BF16)
            xh_slab = consts.tile([128, MC, T * BL], BF16)

            hTf = consts.tile([128, MC, BL], F32)
            nc.vector.memset(hTf, 0.0)
            hTb = consts.tile([128, MC, BL], BF16)
            nc.vector.memset(hTb, 0.0)

            # ---------------- Phase 1: projections ----------------
            with (
                tc.tile_pool(name="proj_in", bufs=2) as pin,
                tc.tile_pool(name="proj_ps", bufs=6, space="PSUM") as pps,
            ):
                for j in range(NBLK):
                    xT_sb = pin.tile([128, KC, PCOLS], BF16, tag="xT_sb")
                    nc.sync.dma_start(
                        xT_sb,
                        xT[:, j * PCOLS : (j + 1) * PCOLS].rearrange(
                            "(c p) n -> p c n", p=128
                        ),
                    )
                    csl = slice(j * PCOLS, (j + 1) * PCOLS)
                    for W_sb, bias_sb, dst in (
                        (Wk_sb, br_sb, xz_slab),
                        (Wu_sb, bur_sb, xh_slab),
                    ):
                        for m in range(MC):
                            ps = pps.tile([128, PCOLS], F32)
                            for k in range(KC):
                                nc.tensor.matmul(
                                    ps,
                                    W_sb[:, k, m * 128 : (m + 1) * 128],
                                    xT_sb[:, k, :],
                                    start=(k == 0),
                                    stop=(k == KC - 1),
                                )
                            # Drain PSUM -> bf16 slab; split ACT / DVE so the
                            # drain never gates the PE stream.
                            if m < 2:
                                nc.scalar.activation(
                                    dst[:, m, csl], ps, ID,
                                    bias=bias_sb[:, m : m + 1],
                                )
                            else:
                                nc.vector.tensor_scalar(
                                    dst[:, m, csl], ps,
                                    bias_sb[:, m : m + 1], None,
                                    mybir.AluOpType.add,
                                )

            # ---------------- Phase 2: recurrence ----------------
            MH = MC // 2  # m-chunks per half
            with (
                tc.tile_pool(name="rec_ps1", bufs=2, space="PSUM") as rps1,
                tc.tile_pool(name="rec_ps2", bufs=2, space="PSUM") as rps2,
                tc.tile_pool(name="rec_tmp", bufs=3) as rtmp,
            ):
                with tc.For_i(0, T * BL, S * BL, staggered_reset=True) as it:
                    # Everything below runs in m-HALVES living in separate
                    # PSUM banks / SBUF tiles so bank-level dependency
                    # tracking lets each half of the chain advance as soon
                    # as its producers finish (software half-pipelining).
                    def mm_bursts(pstiles, W_sb_, rhs_halves, stop_last):
                        # 2x2 burst order: (k-half, m-half) so the first
                        # k-burst starts as soon as rhs half 0 is ready and
                        # each psum half completes one burst early.
                        for kh in range(2):
                            for mh in range(2):
                                for m in range(MH):
                                    for k in range(MH):
                                        kk = kh * MH + k
                                        mm = mh * MH + m
                                        nc.tensor.matmul(
                                            pstiles[mh][:, m, :],
                                            W_sb_[:, kk, mm * 128 : (mm + 1) * 128],
                                            rhs_halves[kh][:, k, :],
                                            start=False,
                                            stop=stop_last and kk == KC - 1,
                                        )

                    def ps_init(pool, slab, s, tagbase):
                        # PSUM init with the step's xz/xh slice via identity
                        # matmul (start=True overwrites the stale bank).
                        tiles = [None, None]
                        for hh in range(2):
                            tiles[hh] = pool.tile(
                                [128, MH, BL], F32,
                                tag=f"{tagbase}{hh}", name=f"{tagbase}{hh}",
                            )
                            nc.tensor.matmul(
                                tiles[hh],
                                id_sb,
                                slab[:, hh * MH : (hh + 1) * MH,
                                     bass.ds(it + s * BL, BL)],
                                start=True,
                                stop=False,
                            )
                        return tiles

                    # chunk head: step 0's mm1 runs from the bf16 state
                    # snapshot saved at the previous chunk boundary.
                    ps1 = ps_init(rps1, xz_slab, 0, "ps1")
                    hTb_h = [hTb[:, 0:MH, :], hTb[:, MH:MC, :]]
                    mm_bursts(ps1, Wr_sb, hTb_h, True)
                    for s in range(S):
                        # chain: sigmoid -> hf (bf16) -> mm2 -> tanh -> e
                        # -> next step's mm1b. The state update h' = A + e
                        # and next mm1's A-part run off the chain:
                        # z1(t+1) = xz(t+1) + A@W_r + e@W_r  (linearity).
                        fT = [None, None]
                        hfh = [None, None]
                        Ab = [None, None]
                        for hh in range(2):
                            msl = slice(hh * MH, (hh + 1) * MH)
                            fT[hh] = rtmp.tile(
                                [128, MH, BL], F32, tag=f"fT{hh}", name=f"fT{hh}"
                            )
                            nc.scalar.activation(fT[hh], ps1[hh], SIG)
                            hfh[hh] = rtmp.tile(
                                [128, MH, BL], BF16, tag=f"hf{hh}", name=f"hf{hh}"
                            )
                            nc.vector.tensor_mul(hfh[hh], hTf[:, msl, :], fT[hh])
                        ps2 = ps_init(rps2, xh_slab, s, "ps2")
                        for hh in range(2):
                            msl = slice(hh * MH, (hh + 1) * MH)
                            Ab[hh] = rtmp.tile(
                                [128, MH, BL], BF16, tag=f"Ab{hh}", name=f"Ab{hh}"
                            )
                            nc.vector.tensor_sub(Ab[hh], hTf[:, msl, :], hfh[hh])
                        mm_bursts(ps2, Wur_sb, hfh, True)
                        ps1n = [None, None]
                        if s < S - 1:
                            ps1n = ps_init(rps1, xz_slab, s + 1, "ps1")
                            mm_bursts(ps1n, Wr_sb, Ab, False)
                        eb = [None, None]
                        for hh in range(2):
                            cT = rtmp.tile(
                                [128, MH, BL], F32, tag=f"cT{hh}", name=f"cT{hh}"
                            )
                            nc.scalar.activation(cT, ps2[hh], TANH)
                            eb[hh] = rtmp.tile(
                                [128, MH, BL], BF16, tag=f"eb{hh}", name=f"eb{hh}"
                            )
                            nc.vector.tensor_mul(eb[hh], cT, fT[hh])
                        if s < S - 1:
                            mm_bursts(ps1n, Wr_sb, eb, True)
                        for hh in range(2):
                            msl = slice(hh * MH, (hh + 1) * MH)
                            nc.vector.tensor_add(hTf[:, msl, :], Ab[hh], eb[hh])
                            if s == S - 1:
                                nc.vector.tensor_add(hTb[:, msl, :], Ab[hh], eb[hh])
                        ps1 = ps1n

            nc.sync.dma_start(hT_out[:, :, :], hTf)

    nc.compile()
    return nc


_NC_CACHE = None


def kernel(x, W_k, W_r, b_r, W_u, W_ur, b_ur):
    global _NC_CACHE, LAST_EXEC_NS
    _install_trace_shim()
    if _NC_CACHE is None:
        _NC_CACHE = _build()
    nc = _NC_CACHE

    bf16 = ml_dtypes.bfloat16
    x = np.asarray(x, dtype=np.float32)
    Wk_b = np.ascontiguousarray(np.asarray(W_k, dtype=np.float32).astype(bf16))
    Wu_b = np.ascontiguousarray(np.asarray(W_u, dtype=np.float32).astype(bf16))
    Wr_b = np.ascontiguousarray(np.asarray(W_r, dtype=np.float32).astype(bf16))
    Wur_b = np.ascontiguousarray(np.asarray(W_ur, dtype=np.float32).astype(bf16))
    br_f = np.ascontiguousarray(np.asarray(b_r, dtype=np.float32))
    bur_f = np.ascontiguousarray(np.asarray(b_ur, dtype=np.float32))
    id_b = np.ascontiguousarray(np.eye(128, dtype=np.float32).astype(bf16))

    in_maps = []
    for c in range(NCORES):
        xc = x[c * BL : (c + 1) * BL]  # [BL, T, D]
        xTc = np.ascontiguousarray(
            xc.transpose(2, 1, 0).reshape(D, T * BL).astype(bf16)
        )
        in_maps.append(
            {
                "xT": xTc,
                "Wk": Wk_b,
                "Wu": Wu_b,
                "Wr": Wr_b,
                "Wur": Wur_b,
                "br": br_f,
                "bur": bur_f,
                "ident": id_b,
            }
        )

    trace = bool(os.environ.get("BASS_TRACE"))
    res = run_bass_kernel_spmd(
        nc, in_maps, core_ids=list(range(NCORES)), trace=trace
    )
    LAST_EXEC_NS = res.exec_time_ns

    out = np.empty((B, U), dtype=np.float32)
    for c in range(NCORES):
        hT = res.results[c]["hT_out"]  # [128, MC, BL]
        out[c * BL : (c + 1) * BL] = hT.transpose(2, 1, 0).reshape(BL, U)
    return out
